# revision 22
# baseline (speedup 1.0000x reference)
"""Trainium2 Bass kernel for prefix-LM CausalSelfAttention.

Problem: B=2, T=2048, C=2048, H=16 heads (hd=128), prefix-LM mask
(bidirectional over first half, causal after), RoPE on q/k.

Sharding over 8 cores: data-parallel on batch (2) x tensor-parallel on
heads (4 heads per core). Each core computes a partial output projection
(its heads' contribution); partials are summed on host.

All matmul operands are bf16 (PE full rate, f32 PSUM accumulation);
only the final output DMA is f32. Per-core dataflow:
  1. qT/kT = W^T @ x^T    [hd*4, T] head-major tiles; x loaded once
     (bf16) and kept resident for both this and the v matmuls.
  2. RoPE (pair-swap matmul + DVE combine) interleaved with
     v = x @ Wv so the PE never idles while the DVE ropes.
  3. Attention per (chunk I, head h), software-pipelined one key-tile
     ahead so exp (ACT) hides under the PE matmuls:
       S'[J] = k[:,J]^T x q[:,I]   (scores transposed, [j,i])
       P'[J] = exp(S' * scale)     (ACT, psum->sbuf bf16)
       mask-multiply for diagonal-crossing tiles (4 static patterns)
       y_psum += v[J,h]^T x P'[J]
       d_psum += ones128^T x P'[J]  -> full [128,512] broadcast rowsum
     yT[:, I] = y_psum * reciprocal_approx_fast(d_psum)
  4. After each chunk I's 4 heads: partial out rows = yT^T @ Wp.

Fully-masked key tiles are skipped (structural sparsity: 44/64 tiles).
"""
import math

import numpy as np

N_HEAD = 16
B = 2
T = 2048
C = 2048
HD = 128
HPC = 4          # heads per core
CL = HPC * HD    # local C = 512
TC = 512         # chunk width (matmul moving free dim / psum bank)
NT = T // TC     # 4 chunks
KT = C // 128    # 16 contraction tiles over C
TT = T // 128    # 16 T tiles
SCALE = 1.0 / math.sqrt(HD)

# Per query-chunk I: list of (J, mask_idx) key tiles to compute.
# mask_idx is None for fully-allowed tiles, else 0..3 selecting the
# static diagonal pattern mask[d][jj, ii] = (ii >= jj + 128*d).
_JLISTS = {
    0: [(j, None) for j in range(8)],
    1: [(j, None) for j in range(8)],
    2: [(j, None) for j in range(8)] + [(8 + d, d) for d in range(4)],
    3: [(j, None) for j in range(12)] + [(12 + d, d) for d in range(4)],
}

_CACHE = {}


def _build_nc():
    import concourse.tile as tile
    import concourse.mybir as mybir
    from concourse import bacc

    f32 = mybir.dt.float32
    bf16 = mybir.dt.bfloat16

    nc = bacc.Bacc(None, target_bir_lowering=False)

    xT = nc.dram_tensor("xT", [C, T], bf16, kind="ExternalInput")
    wqk = nc.dram_tensor("wqk", [C, 2 * CL], bf16, kind="ExternalInput")
    wv = nc.dram_tensor("wv", [C, CL], bf16, kind="ExternalInput")
    wp = nc.dram_tensor("wp", [CL, C], bf16, kind="ExternalInput")
    cosP = nc.dram_tensor("cosP", [HD, T], bf16, kind="ExternalInput")
    sinP = nc.dram_tensor("sinP", [HD, T], bf16, kind="ExternalInput")
    rt = nc.dram_tensor("rt", [HD, HD], bf16, kind="ExternalInput")
    masks = nc.dram_tensor("masks", [4, 128, TC], bf16, kind="ExternalInput")
    ones = nc.dram_tensor("ones", [128, 128], bf16, kind="ExternalInput")
    out = nc.dram_tensor("out", [T, C], bf16, kind="ExternalOutput")

    xT3 = xT.rearrange("(kt p) t -> p kt t", p=128)
    wqk3 = wqk.rearrange("(kt p) m -> p kt m", p=128)
    wv3 = wv.rearrange("(kt p) m -> p kt m", p=128)
    wp3 = wp.rearrange("(kt p) m -> p kt m", p=128)
    masks3 = masks.rearrange("d p n -> p d n")

    Exp = mybir.ActivationFunctionType.Exp

    with tile.TileContext(nc) as tc:
        mpool = tc.alloc_tile_pool(name="misc", bufs=1)
        qk_pool = tc.alloc_tile_pool(name="qkrope", bufs=1)
        tpool = tc.alloc_tile_pool(name="trig", bufs=1, side="right")

        # PE clock warm-up: ~20 dummy matmuls with no DMA deps keep the
        # tensor engine busy (HAM ramps to full clock) while the first
        # weight/x DMAs land.
        warm = mpool.tile([128, 640], bf16, name="warm")
        nc.vector.memset(warm, 0.0)
        ps_w = tc.alloc_tile_pool(name="ps_warm", bufs=1, space="PSUM")
        wps = ps_w.tile([128, TC], f32, tag="wps", name="wps")
        for _ in range(10):
            nc.tensor.matmul(wps, warm[:, :128], warm[:, 128:640],
                             start=True, stop=True)
        ps_w.release()

        rt_sb = mpool.tile([HD, HD], bf16)
        ones_sb = mpool.tile([128, 128], bf16)
        mask_sb = mpool.tile([128, 4, TC], bf16)
        cos_sb = tpool.tile([HD, T], bf16)
        sin_sb = tpool.tile([HD, T], bf16)

        # qkT[m] for m in 0..7: m<4 -> q head m, else k head m-4; [hd, T]
        # (rope outputs later reuse the same slots via identical tags)
        qkT = [qk_pool.tile([128, T], bf16, tag=f"qk{m}", name=f"qk{m}") for m in range(8)]

        # Long-lived pools first (pool release must be LIFO per side):
        # yT / Wp / v live to the end; x / wv / wqk release after stage C.
        y_pool = tc.alloc_tile_pool(name="yT_sb", bufs=1)          # 16K
        yT = [y_pool.tile([128, T], bf16, tag=f"yT{h}", name=f"yT{h}")
              for h in range(HPC)]
        wppool = tc.alloc_tile_pool(name="wp_sb", bufs=1)          # 16K
        v_pool = tc.alloc_tile_pool(name="v_sb", bufs=1)           # 16K ..attn
        v_t = [v_pool.tile([128, CL], bf16, tag=f"v{mt}", name=f"v{mt}")
               for mt in range(TT)]

        # ---- stage A: qT/kT = W_{q,k}^T @ x^T, head-major tiles ----
        # x (bf16) is loaded ONCE, fully resident; stage C reuses it.
        xpool = tc.alloc_tile_pool(name="xt_all", bufs=1)          # 64K A..C
        wvpool = tc.alloc_tile_pool(name="wv_sb", bufs=1)          # 16K ..v
        wpool = tc.alloc_tile_pool(name="wqk_sb", bufs=1)          # 32K A
        ps1 = tc.alloc_tile_pool(name="ps_qk", bufs=4, space="PSUM")
        w_t = []
        x_t = {}
        for k in range(KT):
            wt = wpool.tile([128, 2 * CL], bf16, tag=f"w{k}", name=f"w{k}")
            nc.sync.dma_start(out=wt, in_=wqk3[:, k])
            w_t.append(wt)
            xt = xpool.tile([128, TC], bf16, tag=f"x0_{k}", name=f"x0_{k}")
            nc.scalar.dma_start(out=xt, in_=xT3[:, k, 0:TC])
            x_t[(0, k)] = xt
        nc.sync.dma_start(out=rt_sb, in_=rt[:, :])
        nc.sync.dma_start(out=cos_sb, in_=cosP[:, :])
        nc.sync.dma_start(out=sin_sb, in_=sinP[:, :])
        for n in range(1, NT):
            for k in range(KT):
                xt = xpool.tile([128, TC], bf16, tag=f"x{n}_{k}", name=f"x{n}_{k}")
                nc.scalar.dma_start(out=xt, in_=xT3[:, k, n * TC:(n + 1) * TC])
                x_t[(n, k)] = xt

        # v-phase + attention constants DMA'd early (all overlap stage A)
        wv_t = []
        for k in range(KT):
            wt = wvpool.tile([128, CL], bf16, tag=f"wv{k}", name=f"wv{k}")
            nc.sync.dma_start(out=wt, in_=wv3[:, k])
            wv_t.append(wt)
        nc.sync.dma_start(out=ones_sb, in_=ones[:, :])
        nc.sync.dma_start(out=mask_sb, in_=masks3)
        wp_t = []
        for hk in range(HPC):
            wt = wppool.tile([128, C], bf16, tag=f"wp{hk}", name=f"wp{hk}")
            nc.sync.dma_start(out=wt, in_=wp3[:, hk])
            wp_t.append(wt)

        # n=0 runs k-major over 8 open PSUM banks: each arriving w tile
        # immediately feeds 8 matmuls instead of stalling a whole m-group
        # on the full weight set.
        ps0 = [ps1.tile([128, TC], f32, tag=f"ps_qk{m}", name=f"ps_qk{m}", bufs=1)
               for m in range(8)]
        for k in range(KT):
            for m in range(8):
                nc.tensor.matmul(
                    ps0[m], w_t[k][:, m * 128:(m + 1) * 128], x_t[(0, k)],
                    start=(k == 0), stop=(k == KT - 1),
                )
        for m in range(8):
            nc.vector.tensor_copy(out=qkT[m][:, 0:TC], in_=ps0[m])
        for n in range(1, NT):
            for m in range(8):
                ps = ps1.tile([128, TC], f32, tag=f"ps_qk{m}", name=f"ps_qk{m}",
                              bufs=1)
                for k in range(KT):
                    nc.tensor.matmul(
                        ps, w_t[k][:, m * 128:(m + 1) * 128], x_t[(n, k)],
                        start=(k == 0), stop=(k == KT - 1),
                    )
                nc.vector.tensor_copy(out=qkT[m][:, n * TC:(n + 1) * TC], in_=ps)
        wpool.release()
        ps1.release()

        # ---- stage B+C + chunk-0 attention, interleaved ----
        # RoPE (DVE-heavy) and v = x @ Wv (PE-heavy) interleave so the PE
        # never waits on the DVE rope chain; chunk I=0's attention (which
        # has no proj block to hide its exp latency behind) is woven in as
        # its rope heads and v tiles complete, so its ACT burst runs while
        # the PE still has pure-GEMM work.
        # One unified PSUM pool for everything after stage A:
        #   tag "s" (3 banks): rot + attention score tiles
        #   tag "y" (2 banks): v-GEMM + attention y accumulation
        #   tag "d" (2 banks): softmax denominator + proj accumulation
        rtmp = tc.alloc_tile_pool(name="rope_tmp", bufs=4)
        pp_pool = tc.alloc_tile_pool(name="pp", bufs=5)
        sm_pool = tc.alloc_tile_pool(name="small", bufs=2)
        opool = tc.alloc_tile_pool(name="ostage", bufs=3)
        psA = tc.alloc_tile_pool(name="psA", bufs=2, space="PSUM")

        rope = [None] * 8

        def rope_m(m):
            tmp = []
            for n in range(NT):
                sl = slice(n * TC, (n + 1) * TC)
                ps = psA.tile([128, TC], f32, tag="s", name="ps_rot", bufs=4)
                nc.tensor.matmul(ps, rt_sb, qkT[m][:, sl], start=True, stop=True)
                t1 = rtmp.tile([128, TC], bf16, tag="t1", name="t1")
                t2 = rtmp.tile([128, TC], bf16, tag="t2", name="t2")
                nc.vector.tensor_mul(t1, ps, sin_sb[:, sl])
                nc.vector.tensor_mul(t2, qkT[m][:, sl], cos_sb[:, sl])
                tmp.append((t1, t2))
            # all reads of qkT[m] issued; now write into its slot
            ro = qk_pool.tile([128, T], bf16, tag=f"qk{m}", name=f"rope{m}")
            for n in range(NT):
                sl = slice(n * TC, (n + 1) * TC)
                nc.vector.tensor_add(ro[:, sl], tmp[n][0], tmp[n][1])
            rope[m] = ro

        def v_tile(mt):
            nv = mt // 4
            off = (mt % 4) * 128
            ps = psA.tile([128, CL], f32, tag="y", name="ps_v", bufs=2)
            for k in range(KT):
                nc.tensor.matmul(
                    ps, x_t[(nv, k)][:, off:off + 128],
                    wv_t[k], start=(k == 0), stop=(k == KT - 1),
                )
            nc.scalar.copy(out=v_t[mt], in_=ps)

        def attn_group(I, h, mid=None):
            isl = slice(I * TC, (I + 1) * TC)
            jl = _JLISTS[I]
            nj = len(jl)
            q_h = rope[h]
            k_h = rope[4 + h]
            y_ps = psA.tile([128, TC], f32, tag="y", name="y_ps", bufs=2)
            d_ps = psA.tile([128, TC], f32, tag="d", name="d_ps", bufs=2)

            def emit_S(jidx):
                J, dm = jl[jidx]
                lo = 0 if dm is None else 128 * dm
                csl = slice(lo, TC)
                s_ps = psA.tile([128, TC], f32, tag="s", name="s_ps", bufs=4)
                nc.tensor.matmul(
                    s_ps[:, csl], k_h[:, J * 128:(J + 1) * 128],
                    q_h[:, I * TC + lo:(I + 1) * TC], start=True, stop=True,
                )
                pp = pp_pool.tile([128, TC], bf16, tag="pp", name="pp")
                nc.scalar.activation(out=pp[:, csl], in_=s_ps[:, csl],
                                     func=Exp, scale=SCALE)
                return pp

            # two-tile software pipeline: the PE stream runs
            # S0,S1,S2,AV0,d0,S3,AV1,d1,... so exp[j] (ACT) plus any
            # mask-mul (DVE) hide under two full PE tile-periods.
            pps = [None] * nj
            pps[0] = emit_S(0)
            if nj > 1:
                pps[1] = emit_S(1)
            if mid is not None:
                mid()
            if nj > 2:
                pps[2] = emit_S(2)
            for jidx, (J, dm) in enumerate(jl):
                if jidx + 3 < nj:
                    pps[jidx + 3] = emit_S(jidx + 3)
                pp = pps[jidx]
                lo = 0 if dm is None else 128 * dm
                csl = slice(lo, TC)
                if dm is not None:
                    ppm = pp_pool.tile([128, TC], bf16, tag="ppm",
                                       name="ppm", bufs=2)
                    nc.vector.tensor_mul(ppm[:, csl], pp[:, csl],
                                         mask_sb[:, dm, csl])
                    pp = ppm
                first = jidx == 0
                last = jidx == nj - 1
                nc.tensor.matmul(
                    y_ps[:, csl], v_t[J][:, h * 128:(h + 1) * 128],
                    pp[:, csl], start=first, stop=last,
                )
                nc.tensor.matmul(d_ps[:, csl], ones_sb, pp[:, csl],
                                 start=first, stop=last)
            recip = sm_pool.tile([128, TC], f32, tag="recip", name="recip")
            nc.vector.reciprocal_approx_fast(out=recip, in_=d_ps)
            nc.vector.tensor_mul(yT[h][:, isl], y_ps, recip)

        def emit_proj(Ip, ml):
            # one 128-row out block: out[mt] = sum_h yT[h][:,rows]^T @ Wp[h]
            mt = 4 * Ip + ml
            msl = slice(mt * 128, (mt + 1) * 128)
            ot = opool.tile([128, C], bf16, tag="ot", name="ot")
            for n in range(NT):
                ps = psA.tile([128, TC], f32, tag="d", name="o_ps", bufs=2)
                for hk in range(HPC):
                    nc.tensor.matmul(
                        ps, yT[hk][:, msl], wp_t[hk][:, n * TC:(n + 1) * TC],
                        start=(hk == 0), stop=(hk == HPC - 1),
                    )
                osl = slice(n * TC, (n + 1) * TC)
                if n % 2 == 0:
                    nc.vector.tensor_copy(out=ot[:, osl], in_=ps)
                else:
                    nc.scalar.copy(out=ot[:, osl], in_=ps)
            nc.sync.dma_start(out=out[msl, :], in_=ot)

        # rope heads in (q,k)-pair order so chunk-0 attention heads can
        # start as soon as their pair + the first 8 v tiles are ready;
        # remaining rope/v work rides in the mid slots of chunk-0 groups
        # so every group's first exp hides under real PE work
        def mids(*fns):
            def run():
                for f in fns:
                    f()
            return run

        rope_m(0); v_tile(0); v_tile(1)
        rope_m(4); v_tile(2); v_tile(3)
        rope_m(1); v_tile(4); v_tile(5)
        rope_m(5); v_tile(6); v_tile(7)
        attn_group(0, 0, mids(lambda: rope_m(2), lambda: v_tile(8),
                              lambda: v_tile(9)))
        attn_group(0, 1, mids(lambda: rope_m(6), lambda: v_tile(10),
                              lambda: v_tile(11)))
        attn_group(0, 2, mids(lambda: rope_m(3), lambda: rope_m(7),
                              lambda: v_tile(12), lambda: v_tile(13)))
        attn_group(0, 3, mids(lambda: v_tile(14), lambda: v_tile(15)))

        # ---- chunks 1..3: attention heads with the previous chunk's
        # proj row-blocks in the mid slot (pure-PE work with no ACT
        # dependency, letting the exp queue catch up) ----
        for I in range(1, NT):
            for h in range(HPC):
                attn_group(I, h, (lambda Ip, hh: lambda: emit_proj(Ip, hh))(I - 1, h))
        for ml in range(4):
            emit_proj(NT - 1, ml)

        for p in (psA, opool, sm_pool, pp_pool, rtmp, wvpool, xpool,
                  tpool, v_pool, wppool, y_pool, qk_pool, mpool):
            p.release()
    nc.compile()
    return nc


def _host_prep(x, w_qkv, w_proj, freqs_cis):
    """Build per-core input maps (slicing + layout prep only)."""
    import ml_dtypes
    bf16 = ml_dtypes.bfloat16

    x = np.asarray(x, dtype=np.float32)
    w_qkv = np.asarray(w_qkv, dtype=np.float32)
    w_proj = np.asarray(w_proj, dtype=np.float32)
    fc = np.asarray(freqs_cis, dtype=np.float32)

    xTb = [np.ascontiguousarray(x[b].T).astype(bf16) for b in range(B)]

    cos = fc[:, :, 0].T  # [64, T]
    sin = fc[:, :, 1].T
    cosP = np.repeat(cos, 2, axis=0).astype(bf16)  # [128, T]
    sinP = np.repeat(sin, 2, axis=0).astype(bf16)

    rt = np.zeros((HD, HD), dtype=np.float32)
    for d in range(HD // 2):
        rt[2 * d, 2 * d + 1] = 1.0
        rt[2 * d + 1, 2 * d] = -1.0
    rt = rt.astype(bf16)

    masks = np.zeros((4, 128, TC), dtype=np.float32)
    ii = np.arange(TC)[None, :]
    jj = np.arange(128)[:, None]
    for d in range(4):
        masks[d] = (ii >= jj + 128 * d).astype(np.float32)
    masks = masks.astype(bf16)

    ones = np.ones((128, 128), dtype=bf16)

    in_maps = []
    for core in range(8):
        b = core // 4
        g = core % 4
        qc = w_qkv[:, 512 * g: 512 * (g + 1)]
        kc = w_qkv[:, 2048 + 512 * g: 2048 + 512 * (g + 1)]
        vc = np.ascontiguousarray(w_qkv[:, 4096 + 512 * g: 4096 + 512 * (g + 1)]).astype(bf16)
        wqk_c = np.concatenate([qc, kc], axis=1).astype(bf16)
        wp_c = np.ascontiguousarray(w_proj[512 * g: 512 * (g + 1), :]).astype(bf16)
        in_maps.append({
            "xT": xTb[b],
            "wqk": wqk_c,
            "wv": vc,
            "wp": wp_c,
            "cosP": cosP,
            "sinP": sinP,
            "rt": rt,
            "masks": masks,
            "ones": ones,
        })
    return in_maps


def _get_nc():
    if "nc" not in _CACHE:
        _CACHE["nc"] = _build_nc()
    return _CACHE["nc"]


def kernel(x, w_qkv, w_proj, freqs_cis, attn_mask, _trace=False):
    from concourse.bass_utils import run_bass_kernel_spmd

    in_maps = _host_prep(x, w_qkv, w_proj, freqs_cis)
    nc = _get_nc()
    res = run_bass_kernel_spmd(
        nc, in_maps, core_ids=list(range(8)), trace=_trace,
    )
    outs = [r["out"].astype(np.float64) for r in res.results]
    full = np.stack([
        outs[0] + outs[1] + outs[2] + outs[3],
        outs[4] + outs[5] + outs[6] + outs[7],
    ]).astype(np.float32)
    if _trace:
        kernel._last_results = res
    return full


# revision 23
# speedup vs baseline: 1.0076x; 1.0076x over previous
"""Trainium2 Bass kernel for prefix-LM CausalSelfAttention.

Problem: B=2, T=2048, C=2048, H=16 heads (hd=128), prefix-LM mask
(bidirectional over first half, causal after), RoPE on q/k.

Sharding over 8 cores: data-parallel on batch (2) x tensor-parallel on
heads (4 heads per core). Each core computes a partial output projection
(its heads' contribution); partials are summed on host.

All matmul operands are bf16 (PE full rate, f32 PSUM accumulation);
only the final output DMA is f32. Per-core dataflow:
  1. qT/kT = W^T @ x^T    [hd*4, T] head-major tiles; x loaded once
     (bf16) and kept resident for both this and the v matmuls.
  2. RoPE (pair-swap matmul + DVE combine) interleaved with
     v = x @ Wv so the PE never idles while the DVE ropes.
  3. Attention per (chunk I, head h), software-pipelined one key-tile
     ahead so exp (ACT) hides under the PE matmuls:
       S'[J] = k[:,J]^T x q[:,I]   (scores transposed, [j,i])
       P'[J] = exp(S' * scale)     (ACT, psum->sbuf bf16)
       mask-multiply for diagonal-crossing tiles (4 static patterns)
       y_psum += v[J,h]^T x P'[J]
       d_psum += ones128^T x P'[J]  -> full [128,512] broadcast rowsum
     yT[:, I] = y_psum * reciprocal_approx_fast(d_psum)
  4. After each chunk I's 4 heads: partial out rows = yT^T @ Wp.

Fully-masked key tiles are skipped (structural sparsity: 44/64 tiles).
"""
import math

import numpy as np

N_HEAD = 16
B = 2
T = 2048
C = 2048
HD = 128
HPC = 4          # heads per core
CL = HPC * HD    # local C = 512
TC = 512         # chunk width (matmul moving free dim / psum bank)
NT = T // TC     # 4 chunks
KT = C // 128    # 16 contraction tiles over C
TT = T // 128    # 16 T tiles
SCALE = 1.0 / math.sqrt(HD)

# Per query-chunk I: list of (J, mask_idx) key tiles to compute.
# mask_idx is None for fully-allowed tiles, else 0..3 selecting the
# static diagonal pattern mask[d][jj, ii] = (ii >= jj + 128*d).
_JLISTS = {
    0: [(j, None) for j in range(8)],
    1: [(j, None) for j in range(8)],
    # diagonal (masked, range-restricted) tiles FIRST: their short
    # matmuls and long exp->mask chains hide under the group's mid work,
    # and the group tail is uniform full tiles (keeps the software
    # pipeline's slack constant). Tile 8+0 / 12+0 spans the full column
    # range, so it legally carries the start=True PSUM init.
    2: [(8 + d, d) for d in range(4)] + [(j, None) for j in range(8)],
    3: [(12 + d, d) for d in range(4)] + [(j, None) for j in range(12)],
}

_CACHE = {}


def _build_nc():
    import concourse.tile as tile
    import concourse.mybir as mybir
    from concourse import bacc

    f32 = mybir.dt.float32
    bf16 = mybir.dt.bfloat16

    nc = bacc.Bacc(None, target_bir_lowering=False)

    xT = nc.dram_tensor("xT", [C, T], bf16, kind="ExternalInput")
    wqk = nc.dram_tensor("wqk", [C, 2 * CL], bf16, kind="ExternalInput")
    wv = nc.dram_tensor("wv", [C, CL], bf16, kind="ExternalInput")
    wp = nc.dram_tensor("wp", [CL, C], bf16, kind="ExternalInput")
    cosP = nc.dram_tensor("cosP", [HD, T], bf16, kind="ExternalInput")
    sinP = nc.dram_tensor("sinP", [HD, T], bf16, kind="ExternalInput")
    rt = nc.dram_tensor("rt", [HD, HD], bf16, kind="ExternalInput")
    masks = nc.dram_tensor("masks", [4, 128, TC], bf16, kind="ExternalInput")
    ones = nc.dram_tensor("ones", [128, 128], bf16, kind="ExternalInput")
    out = nc.dram_tensor("out", [T, C], bf16, kind="ExternalOutput")

    xT3 = xT.rearrange("(kt p) t -> p kt t", p=128)
    wqk3 = wqk.rearrange("(kt p) m -> p kt m", p=128)
    wv3 = wv.rearrange("(kt p) m -> p kt m", p=128)
    wp3 = wp.rearrange("(kt p) m -> p kt m", p=128)
    masks3 = masks.rearrange("d p n -> p d n")

    Exp = mybir.ActivationFunctionType.Exp

    with tile.TileContext(nc) as tc:
        mpool = tc.alloc_tile_pool(name="misc", bufs=1)
        qk_pool = tc.alloc_tile_pool(name="qkrope", bufs=1)
        tpool = tc.alloc_tile_pool(name="trig", bufs=1, side="right")

        # PE clock warm-up: ~20 dummy matmuls with no DMA deps keep the
        # tensor engine busy (HAM ramps to full clock) while the first
        # weight/x DMAs land.
        warm = mpool.tile([128, 640], bf16, name="warm")
        nc.vector.memset(warm, 0.0)
        ps_w = tc.alloc_tile_pool(name="ps_warm", bufs=1, space="PSUM")
        wps = ps_w.tile([128, TC], f32, tag="wps", name="wps")
        for _ in range(10):
            nc.tensor.matmul(wps, warm[:, :128], warm[:, 128:640],
                             start=True, stop=True)
        ps_w.release()

        rt_sb = mpool.tile([HD, HD], bf16)
        ones_sb = mpool.tile([128, 128], bf16)
        mask_sb = mpool.tile([128, 4, TC], bf16)
        cos_sb = tpool.tile([HD, T], bf16)
        sin_sb = tpool.tile([HD, T], bf16)

        # qkT[m] for m in 0..7: m<4 -> q head m, else k head m-4; [hd, T]
        # (rope outputs later reuse the same slots via identical tags)
        qkT = [qk_pool.tile([128, T], bf16, tag=f"qk{m}", name=f"qk{m}") for m in range(8)]

        # Long-lived pools first (pool release must be LIFO per side):
        # yT / Wp / v live to the end; x / wv / wqk release after stage C.
        y_pool = tc.alloc_tile_pool(name="yT_sb", bufs=1)          # 16K
        yT = [y_pool.tile([128, T], bf16, tag=f"yT{h}", name=f"yT{h}")
              for h in range(HPC)]
        wppool = tc.alloc_tile_pool(name="wp_sb", bufs=1)          # 16K
        v_pool = tc.alloc_tile_pool(name="v_sb", bufs=1)           # 16K ..attn
        v_t = [v_pool.tile([128, CL], bf16, tag=f"v{mt}", name=f"v{mt}")
               for mt in range(TT)]

        # ---- stage A: qT/kT = W_{q,k}^T @ x^T, head-major tiles ----
        # x (bf16) is loaded ONCE, fully resident; stage C reuses it.
        xpool = tc.alloc_tile_pool(name="xt_all", bufs=1)          # 64K A..C
        wvpool = tc.alloc_tile_pool(name="wv_sb", bufs=1)          # 16K ..v
        wpool = tc.alloc_tile_pool(name="wqk_sb", bufs=1)          # 32K A
        ps1 = tc.alloc_tile_pool(name="ps_qk", bufs=4, space="PSUM")
        w_t = []
        x_t = {}
        for k in range(KT):
            wt = wpool.tile([128, 2 * CL], bf16, tag=f"w{k}", name=f"w{k}")
            nc.sync.dma_start(out=wt, in_=wqk3[:, k])
            w_t.append(wt)
            xt = xpool.tile([128, TC], bf16, tag=f"x0_{k}", name=f"x0_{k}")
            nc.scalar.dma_start(out=xt, in_=xT3[:, k, 0:TC])
            x_t[(0, k)] = xt
        nc.sync.dma_start(out=rt_sb, in_=rt[:, :])
        nc.sync.dma_start(out=cos_sb, in_=cosP[:, :])
        nc.sync.dma_start(out=sin_sb, in_=sinP[:, :])
        for n in range(1, NT):
            for k in range(KT):
                xt = xpool.tile([128, TC], bf16, tag=f"x{n}_{k}", name=f"x{n}_{k}")
                nc.scalar.dma_start(out=xt, in_=xT3[:, k, n * TC:(n + 1) * TC])
                x_t[(n, k)] = xt

        # v-phase + attention constants DMA'd early (all overlap stage A)
        wv_t = []
        for k in range(KT):
            wt = wvpool.tile([128, CL], bf16, tag=f"wv{k}", name=f"wv{k}")
            nc.sync.dma_start(out=wt, in_=wv3[:, k])
            wv_t.append(wt)
        nc.sync.dma_start(out=ones_sb, in_=ones[:, :])
        nc.sync.dma_start(out=mask_sb, in_=masks3)
        wp_t = []
        for hk in range(HPC):
            wt = wppool.tile([128, C], bf16, tag=f"wp{hk}", name=f"wp{hk}")
            nc.sync.dma_start(out=wt, in_=wp3[:, hk])
            wp_t.append(wt)

        # n=0 runs k-major over 8 open PSUM banks: each arriving w tile
        # immediately feeds 8 matmuls instead of stalling a whole m-group
        # on the full weight set.
        ps0 = [ps1.tile([128, TC], f32, tag=f"ps_qk{m}", name=f"ps_qk{m}", bufs=1)
               for m in range(8)]
        for k in range(KT):
            for m in range(8):
                nc.tensor.matmul(
                    ps0[m], w_t[k][:, m * 128:(m + 1) * 128], x_t[(0, k)],
                    start=(k == 0), stop=(k == KT - 1),
                )
        for m in range(8):
            nc.vector.tensor_copy(out=qkT[m][:, 0:TC], in_=ps0[m])
        for n in range(1, NT):
            for m in range(8):
                ps = ps1.tile([128, TC], f32, tag=f"ps_qk{m}", name=f"ps_qk{m}",
                              bufs=1)
                for k in range(KT):
                    nc.tensor.matmul(
                        ps, w_t[k][:, m * 128:(m + 1) * 128], x_t[(n, k)],
                        start=(k == 0), stop=(k == KT - 1),
                    )
                nc.vector.tensor_copy(out=qkT[m][:, n * TC:(n + 1) * TC], in_=ps)
        wpool.release()
        ps1.release()

        # ---- stage B+C + chunk-0 attention, interleaved ----
        # RoPE (DVE-heavy) and v = x @ Wv (PE-heavy) interleave so the PE
        # never waits on the DVE rope chain; chunk I=0's attention (which
        # has no proj block to hide its exp latency behind) is woven in as
        # its rope heads and v tiles complete, so its ACT burst runs while
        # the PE still has pure-GEMM work.
        # One unified PSUM pool for everything after stage A:
        #   tag "s" (3 banks): rot + attention score tiles
        #   tag "y" (2 banks): v-GEMM + attention y accumulation
        #   tag "d" (2 banks): softmax denominator + proj accumulation
        rtmp = tc.alloc_tile_pool(name="rope_tmp", bufs=4)
        pp_pool = tc.alloc_tile_pool(name="pp", bufs=5)
        sm_pool = tc.alloc_tile_pool(name="small", bufs=2)
        opool = tc.alloc_tile_pool(name="ostage", bufs=3)
        psA = tc.alloc_tile_pool(name="psA", bufs=2, space="PSUM")

        rope = [None] * 8

        def rope_m(m):
            tmp = []
            for n in range(NT):
                sl = slice(n * TC, (n + 1) * TC)
                ps = psA.tile([128, TC], f32, tag="s", name="ps_rot", bufs=4)
                nc.tensor.matmul(ps, rt_sb, qkT[m][:, sl], start=True, stop=True)
                t1 = rtmp.tile([128, TC], bf16, tag="t1", name="t1")
                t2 = rtmp.tile([128, TC], bf16, tag="t2", name="t2")
                nc.vector.tensor_mul(t1, ps, sin_sb[:, sl])
                nc.vector.tensor_mul(t2, qkT[m][:, sl], cos_sb[:, sl])
                tmp.append((t1, t2))
            # all reads of qkT[m] issued; now write into its slot
            ro = qk_pool.tile([128, T], bf16, tag=f"qk{m}", name=f"rope{m}")
            for n in range(NT):
                sl = slice(n * TC, (n + 1) * TC)
                nc.vector.tensor_add(ro[:, sl], tmp[n][0], tmp[n][1])
            rope[m] = ro

        def v_tile(mt):
            nv = mt // 4
            off = (mt % 4) * 128
            ps = psA.tile([128, CL], f32, tag="y", name="ps_v", bufs=2)
            for k in range(KT):
                nc.tensor.matmul(
                    ps, x_t[(nv, k)][:, off:off + 128],
                    wv_t[k], start=(k == 0), stop=(k == KT - 1),
                )
            nc.scalar.copy(out=v_t[mt], in_=ps)

        def attn_group(I, h, mid=None):
            isl = slice(I * TC, (I + 1) * TC)
            jl = _JLISTS[I]
            nj = len(jl)
            q_h = rope[h]
            k_h = rope[4 + h]
            y_ps = psA.tile([128, TC], f32, tag="y", name="y_ps", bufs=2)
            d_ps = psA.tile([128, TC], f32, tag="d", name="d_ps", bufs=2)

            def emit_S(jidx):
                J, dm = jl[jidx]
                lo = 0 if dm is None else 128 * dm
                csl = slice(lo, TC)
                s_ps = psA.tile([128, TC], f32, tag="s", name="s_ps", bufs=4)
                nc.tensor.matmul(
                    s_ps[:, csl], k_h[:, J * 128:(J + 1) * 128],
                    q_h[:, I * TC + lo:(I + 1) * TC], start=True, stop=True,
                )
                pp = pp_pool.tile([128, TC], bf16, tag="pp", name="pp")
                nc.scalar.activation(out=pp[:, csl], in_=s_ps[:, csl],
                                     func=Exp, scale=SCALE)
                return pp

            # two-tile software pipeline: the PE stream runs
            # S0,S1,S2,AV0,d0,S3,AV1,d1,... so exp[j] (ACT) plus any
            # mask-mul (DVE) hide under two full PE tile-periods.
            pps = [None] * nj
            pps[0] = emit_S(0)
            if nj > 1:
                pps[1] = emit_S(1)
            if mid is not None:
                mid()
            if nj > 2:
                pps[2] = emit_S(2)
            for jidx, (J, dm) in enumerate(jl):
                if jidx + 3 < nj:
                    pps[jidx + 3] = emit_S(jidx + 3)
                pp = pps[jidx]
                lo = 0 if dm is None else 128 * dm
                csl = slice(lo, TC)
                if dm is not None:
                    ppm = pp_pool.tile([128, TC], bf16, tag="ppm",
                                       name="ppm", bufs=2)
                    nc.vector.tensor_mul(ppm[:, csl], pp[:, csl],
                                         mask_sb[:, dm, csl])
                    pp = ppm
                first = jidx == 0
                last = jidx == nj - 1
                nc.tensor.matmul(
                    y_ps[:, csl], v_t[J][:, h * 128:(h + 1) * 128],
                    pp[:, csl], start=first, stop=last,
                )
                nc.tensor.matmul(d_ps[:, csl], ones_sb, pp[:, csl],
                                 start=first, stop=last)
            recip = sm_pool.tile([128, TC], f32, tag="recip", name="recip")
            nc.vector.reciprocal_approx_fast(out=recip, in_=d_ps)
            nc.vector.tensor_mul(yT[h][:, isl], y_ps, recip)

        def emit_proj(Ip, ml):
            # one 128-row out block: out[mt] = sum_h yT[h][:,rows]^T @ Wp[h]
            mt = 4 * Ip + ml
            msl = slice(mt * 128, (mt + 1) * 128)
            ot = opool.tile([128, C], bf16, tag="ot", name="ot")
            for n in range(NT):
                ps = psA.tile([128, TC], f32, tag="d", name="o_ps", bufs=2)
                for hk in range(HPC):
                    nc.tensor.matmul(
                        ps, yT[hk][:, msl], wp_t[hk][:, n * TC:(n + 1) * TC],
                        start=(hk == 0), stop=(hk == HPC - 1),
                    )
                osl = slice(n * TC, (n + 1) * TC)
                if n % 2 == 0:
                    nc.vector.tensor_copy(out=ot[:, osl], in_=ps)
                else:
                    nc.scalar.copy(out=ot[:, osl], in_=ps)
            nc.sync.dma_start(out=out[msl, :], in_=ot)

        # rope heads in (q,k)-pair order so chunk-0 attention heads can
        # start as soon as their pair + the first 8 v tiles are ready;
        # remaining rope/v work rides in the mid slots of chunk-0 groups
        # so every group's first exp hides under real PE work
        def mids(*fns):
            def run():
                for f in fns:
                    f()
            return run

        rope_m(0); v_tile(0); v_tile(1)
        rope_m(4); v_tile(2); v_tile(3)
        rope_m(1); v_tile(4); v_tile(5)
        rope_m(5); v_tile(6); v_tile(7)
        attn_group(0, 0, mids(lambda: rope_m(2), lambda: v_tile(8),
                              lambda: v_tile(9)))
        attn_group(0, 1, mids(lambda: rope_m(6), lambda: v_tile(10),
                              lambda: v_tile(11)))
        attn_group(0, 2, mids(lambda: rope_m(3), lambda: rope_m(7),
                              lambda: v_tile(12), lambda: v_tile(13)))
        attn_group(0, 3, mids(lambda: v_tile(14), lambda: v_tile(15)))

        # ---- chunks 1..3: attention heads with the previous chunk's
        # proj row-blocks in the mid slot (pure-PE work with no ACT
        # dependency, letting the exp queue catch up) ----
        for I in range(1, NT):
            for h in range(HPC):
                attn_group(I, h, (lambda Ip, hh: lambda: emit_proj(Ip, hh))(I - 1, h))
        for ml in range(4):
            emit_proj(NT - 1, ml)

        for p in (psA, opool, sm_pool, pp_pool, rtmp, wvpool, xpool,
                  tpool, v_pool, wppool, y_pool, qk_pool, mpool):
            p.release()
    nc.compile()
    return nc


def _host_prep(x, w_qkv, w_proj, freqs_cis):
    """Build per-core input maps (slicing + layout prep only)."""
    import ml_dtypes
    bf16 = ml_dtypes.bfloat16

    x = np.asarray(x, dtype=np.float32)
    w_qkv = np.asarray(w_qkv, dtype=np.float32)
    w_proj = np.asarray(w_proj, dtype=np.float32)
    fc = np.asarray(freqs_cis, dtype=np.float32)

    xTb = [np.ascontiguousarray(x[b].T).astype(bf16) for b in range(B)]

    cos = fc[:, :, 0].T  # [64, T]
    sin = fc[:, :, 1].T
    cosP = np.repeat(cos, 2, axis=0).astype(bf16)  # [128, T]
    sinP = np.repeat(sin, 2, axis=0).astype(bf16)

    rt = np.zeros((HD, HD), dtype=np.float32)
    for d in range(HD // 2):
        rt[2 * d, 2 * d + 1] = 1.0
        rt[2 * d + 1, 2 * d] = -1.0
    rt = rt.astype(bf16)

    masks = np.zeros((4, 128, TC), dtype=np.float32)
    ii = np.arange(TC)[None, :]
    jj = np.arange(128)[:, None]
    for d in range(4):
        masks[d] = (ii >= jj + 128 * d).astype(np.float32)
    masks = masks.astype(bf16)

    ones = np.ones((128, 128), dtype=bf16)

    in_maps = []
    for core in range(8):
        b = core // 4
        g = core % 4
        qc = w_qkv[:, 512 * g: 512 * (g + 1)]
        kc = w_qkv[:, 2048 + 512 * g: 2048 + 512 * (g + 1)]
        vc = np.ascontiguousarray(w_qkv[:, 4096 + 512 * g: 4096 + 512 * (g + 1)]).astype(bf16)
        wqk_c = np.concatenate([qc, kc], axis=1).astype(bf16)
        wp_c = np.ascontiguousarray(w_proj[512 * g: 512 * (g + 1), :]).astype(bf16)
        in_maps.append({
            "xT": xTb[b],
            "wqk": wqk_c,
            "wv": vc,
            "wp": wp_c,
            "cosP": cosP,
            "sinP": sinP,
            "rt": rt,
            "masks": masks,
            "ones": ones,
        })
    return in_maps


def _get_nc():
    if "nc" not in _CACHE:
        _CACHE["nc"] = _build_nc()
    return _CACHE["nc"]


def kernel(x, w_qkv, w_proj, freqs_cis, attn_mask, _trace=False):
    from concourse.bass_utils import run_bass_kernel_spmd

    in_maps = _host_prep(x, w_qkv, w_proj, freqs_cis)
    nc = _get_nc()
    res = run_bass_kernel_spmd(
        nc, in_maps, core_ids=list(range(8)), trace=_trace,
    )
    outs = [r["out"].astype(np.float64) for r in res.results]
    full = np.stack([
        outs[0] + outs[1] + outs[2] + outs[3],
        outs[4] + outs[5] + outs[6] + outs[7],
    ]).astype(np.float32)
    if _trace:
        kernel._last_results = res
    return full
